# revision 8
# baseline (speedup 1.0000x reference)
"""CrossShareUnit TRN2 kernel — 8-core data-parallel Bass/Tile implementation.

Math (B=8192, L=1, D=1024, K=8):
    G_vec_a = (h_a @ W1.T + b1) @ W2.T + b2                     # [B, D]
    arg[b,k] = sum_{d,e} h_a[b,d] * G[d,k,e] * h_b[b,e]
    c[b]     = sum_k tanh(arg[b,k])
    attn     = softmax(c[b] * G_vec_a[b,:], axis=-1)
    out      = h_a + attn * h_a
for both directions (l->m with G_l_m, m->l with G_m_l).

Per core (batch shard of 1024):
  - s = h_aT.T @ G2 (G2 = reshape(G, [1024, 8192])) on TensorE as a 3-pass
    bf16 hi/lo split (hi*hi + hi*lo + lo*hi), exact to ~2^-17 relative.
  - arg via fused DVE tensor_tensor_reduce of the PSUM s-tile against the
    fp32 h_b rows; tanh + sum_k in one ScalarE activation with accum_out.
  - MLP: fc1 as 3-pass bf16 (reusing the h_aT splits), fc2 in plain fp32.
  - softmax fused on the fc2 PSUM tile (no max-subtract needed: |logits|<=10),
    residual via one scalar_tensor_tensor.
"""
import os
import sys

for _p in ("/opt/trn_rl_repo", "/root/.axon_site/_ro/trn_rl_repo"):
    if os.path.isdir(_p) and _p not in sys.path:
        sys.path.append(_p)

from contextlib import ExitStack

import numpy as np
import ml_dtypes

import concourse.bass as bass
import concourse.tile as tile
from concourse import bacc, mybir, bass_utils

BF16 = mybir.dt.bfloat16
F32 = mybir.dt.float32
AF = mybir.ActivationFunctionType
OP = mybir.AluOpType

B, D, K = 8192, 1024, 8
NC_CORES = 8
BC = B // NC_CORES          # 1024 batch rows per core
NB = BC // 128              # 8 batch chunks of 128
DC = D // 128               # 8 contraction chunks
KD = K * D                  # 8192
NN = KD // 512              # 16 G column blocks of 512
H_L, H_M = 256, 128         # fc1 hidden sizes


def _emit_fc1(nc, fc1ps, w1_hi, w1_lo, aT_hi, aT_lo, h1t, b1_t, n_hc):
    """h1T[h, b] = W1T.T @ h_aT + b1, 3-pass bf16. h1t: [128, n_hc, BC]."""
    for hc in range(n_hc):
        for bh in range(BC // 512):
            p = fc1ps.tile([128, 512], F32)
            first = True
            for c in range(DC):
                w_hi = w1_hi[:, c, hc * 128:(hc + 1) * 128]
                w_lo = w1_lo[:, c, hc * 128:(hc + 1) * 128]
                a_hi = aT_hi[:, c, bh * 512:(bh + 1) * 512]
                a_lo = aT_lo[:, c, bh * 512:(bh + 1) * 512]
                last = c == DC - 1
                nc.tensor.matmul(p, w_hi, a_hi, start=first, stop=False)
                nc.tensor.matmul(p, w_hi, a_lo, start=False, stop=False)
                nc.tensor.matmul(p, w_lo, a_hi, start=False, stop=last)
                first = False
            # psum + per-partition bias -> SBUF
            nc.scalar.activation(
                h1t[:, hc, bh * 512:(bh + 1) * 512], p, AF.Identity,
                bias=b1_t[:, hc:hc + 1], scale=1.0)


def build_program():
    nc = bacc.Bacc("TRN2", target_bir_lowering=False, debug=False,
                   num_devices=NC_CORES)

    def din(name, shape, dt):
        return nc.dram_tensor(name, shape, dt, kind="ExternalInput").ap()

    def dout(name, shape):
        return nc.dram_tensor(name, shape, F32, kind="ExternalOutput").ap()

    # per-core sharded inputs
    haT_hi_d = din("haT_hi", [D, BC], BF16)
    haT_lo_d = din("haT_lo", [D, BC], BF16)
    hmT_hi_d = din("hmT_hi", [D, BC], BF16)
    hmT_lo_d = din("hmT_lo", [D, BC], BF16)
    ha_d = din("ha", [BC, D], F32)
    hm_d = din("hm", [BC, D], F32)
    # replicated
    gl_hi_d = din("gl_hi", [D, KD], BF16)
    gl_lo_d = din("gl_lo", [D, KD], BF16)
    gm_hi_d = din("gm_hi", [D, KD], BF16)
    gm_lo_d = din("gm_lo", [D, KD], BF16)
    w1l_hi_d = din("w1l_hi", [D, H_L], BF16)
    w1l_lo_d = din("w1l_lo", [D, H_L], BF16)
    w1m_hi_d = din("w1m_hi", [D, H_M], BF16)
    w1m_lo_d = din("w1m_lo", [D, H_M], BF16)
    w2l_d = din("w2l", [H_L, D], F32)
    w2m_d = din("w2m", [H_M, D], F32)
    b1l_d = din("b1l", [128, 2], F32)
    b1m_d = din("b1m", [128, 1], F32)
    b2_d = din("b2", [2, D], F32)

    l_out_d = dout("l_out", [BC, D])
    m_out_d = dout("m_out", [BC, D])
    l_attn_d = dout("l_attn", [BC, D])
    m_attn_d = dout("m_attn", [BC, D])

    with tile.TileContext(nc) as tc, ExitStack() as ctx:
        resid = ctx.enter_context(tc.tile_pool(name="resid", bufs=1))
        small = ctx.enter_context(tc.tile_pool(name="small", bufs=2))

        # ---- resident tensors ----
        def load3(dst, src, inner):
            nc.sync.dma_start(out=dst, in_=src.rearrange("(c p) n -> p c n", p=128))

        haT_hi = resid.tile([128, DC, BC], BF16, tag="haT_hi")
        haT_lo = resid.tile([128, DC, BC], BF16, tag="haT_lo")
        hmT_hi = resid.tile([128, DC, BC], BF16, tag="hmT_hi")
        hmT_lo = resid.tile([128, DC, BC], BF16, tag="hmT_lo")
        ha_t = resid.tile([128, NB, D], F32, tag="ha")
        hm_t = resid.tile([128, NB, D], F32, tag="hm")
        for dst, src in ((haT_hi, haT_hi_d), (haT_lo, haT_lo_d),
                         (hmT_hi, hmT_hi_d), (hmT_lo, hmT_lo_d),
                         (ha_t, ha_d), (hm_t, hm_d)):
            load3(dst, src, None)

        w2l_t = resid.tile([128, H_L // 128, D], F32, tag="w2l")
        w2m_t = resid.tile([128, H_M // 128, D], F32, tag="w2m")
        load3(w2l_t, w2l_d, None)
        load3(w2m_t, w2m_d, None)
        b1l_t = resid.tile([128, 2], F32, tag="b1l")
        b1m_t = resid.tile([128, 1], F32, tag="b1m")
        nc.sync.dma_start(out=b1l_t, in_=b1l_d)
        nc.sync.dma_start(out=b1m_t, in_=b1m_d)
        # b2 rows broadcast across partitions (DVE can't take stride-0 APs)
        b2bc = resid.tile([128, 2, D], F32, tag="b2bc")
        for r in range(2):
            row = b2_d[r:r + 1, :]
            src = bass.AP(tensor=row.tensor, offset=row.offset,
                          ap=[[0, 128]] + [list(a) for a in row.ap[1:]])
            nc.sync.dma_start(out=b2bc[:, r, :], in_=src)

        h1tl = resid.tile([128, H_L // 128, BC], F32, tag="h1tl")
        h1tm = resid.tile([128, H_M // 128, BC], F32, tag="h1tm")
        arg_l = resid.tile([128, NB, K, 2], F32, tag="arg_l")
        arg_m = resid.tile([128, NB, K, 2], F32, tag="arg_m")
        c_l = resid.tile([128, NB], F32, tag="c_l")
        c_m = resid.tile([128, NB], F32, tag="c_m")

        # ---- fc1 for both directions (3-pass bf16) ----
        with tc.tile_pool(name="w1pool", bufs=1) as w1p, \
             tc.tile_pool(name="fc1ps", bufs=2, space="PSUM") as fc1ps:
            w1l_hi = w1p.tile([128, DC, H_L], BF16, tag="w1l_hi")
            w1l_lo = w1p.tile([128, DC, H_L], BF16, tag="w1l_lo")
            w1m_hi = w1p.tile([128, DC, H_M], BF16, tag="w1m_hi")
            w1m_lo = w1p.tile([128, DC, H_M], BF16, tag="w1m_lo")
            load3(w1l_hi, w1l_hi_d, None)
            load3(w1l_lo, w1l_lo_d, None)
            load3(w1m_hi, w1m_hi_d, None)
            load3(w1m_lo, w1m_lo_d, None)
            _emit_fc1(nc, fc1ps, w1l_hi, w1l_lo, haT_hi, haT_lo, h1tl, b1l_t, 2)
            _emit_fc1(nc, fc1ps, w1m_hi, w1m_lo, hmT_hi, hmT_lo, h1tm, b1m_t, 1)

        gpool = ctx.enter_context(tc.tile_pool(name="gpool", bufs=2))
        spsum = ctx.enter_context(tc.tile_pool(name="spsum", bufs=3, space="PSUM"))
        gvps = ctx.enter_context(tc.tile_pool(name="gvps", bufs=2, space="PSUM"))

        def phase1(g_hi_d, g_lo_d, aT_hi, aT_lo, hb_t, arg_t):
            """s = aT.T @ G2 (3-pass);  arg[b, k, j] = <s_tile, hb_slice>."""
            for n in range(NN):
                k, j = n // 2, n % 2
                ghi = gpool.tile([128, DC, 512], BF16, tag="ghi")
                glo = gpool.tile([128, DC, 512], BF16, tag="glo")
                src_hi = g_hi_d[:, n * 512:(n + 1) * 512]
                src_lo = g_lo_d[:, n * 512:(n + 1) * 512]
                nc.sync.dma_start(out=ghi, in_=src_hi.rearrange("(c p) n -> p c n", p=128))
                nc.sync.dma_start(out=glo, in_=src_lo.rearrange("(c p) n -> p c n", p=128))
                for bc in range(NB):
                    p = spsum.tile([128, 512], F32)
                    first = True
                    for c in range(DC):
                        a_hi = aT_hi[:, c, bc * 128:(bc + 1) * 128]
                        a_lo = aT_lo[:, c, bc * 128:(bc + 1) * 128]
                        last = c == DC - 1
                        nc.tensor.matmul(p, a_hi, ghi[:, c, :], start=first, stop=False)
                        nc.tensor.matmul(p, a_hi, glo[:, c, :], start=False, stop=False)
                        nc.tensor.matmul(p, a_lo, ghi[:, c, :], start=False, stop=last)
                        first = False
                    # arg[b, k, j] = sum_e s[b, e] * h_b[b, e]; product scratch
                    # written back over the psum tile (no SBUF scratch needed).
                    # scalar_tensor_tensor, NOT tensor_tensor_reduce: the TTR
                    # opcode hard-crashes the exec unit on this hw/ucode.
                    nc.vector.scalar_tensor_tensor(
                        out=p, in0=p, scalar=1.0,
                        in1=hb_t[:, bc, j * 512:(j + 1) * 512],
                        op0=OP.mult, op1=OP.mult,
                        accum_out=arg_t[:, bc, k, j:j + 1])

        def phase2(arg_t, c_t, h1t, n_hc, w2_t, b2_row, ha_nat, out_d, attn_d):
            for bc in range(NB):
                targ = small.tile([128, K], F32, tag="targ")
                nc.vector.tensor_reduce(
                    out=targ, in_=arg_t[:, bc], axis=mybir.AxisListType.X,
                    op=OP.add)
                tscr = small.tile([128, K], F32, tag="tscr")
                nc.scalar.activation(tscr, targ, AF.Tanh,
                                     accum_out=c_t[:, bc:bc + 1])
                # fc2: Gv[b, d] = h1T.T @ W2T  (fp32), one 2-bank psum row
                gv = gvps.tile([128, D], F32)
                for dh in range(2):
                    for hc in range(n_hc):
                        nc.tensor.matmul(
                            gv[:, dh * 512:(dh + 1) * 512],
                            h1t[:, hc, bc * 128:(bc + 1) * 128],
                            w2_t[:, hc, dh * 512:(dh + 1) * 512],
                            start=(hc == 0), stop=(hc == n_hc - 1))
                nc.vector.tensor_tensor(out=gv, in0=gv, in1=b2bc[:, b2_row, :],
                                        op=OP.add)
                expt = small.tile([128, D], F32, tag="expt", bufs=1)
                sume = small.tile([128, 1], F32, tag="sume")
                nc.scalar.activation(expt, gv, AF.Exp,
                                     scale=c_t[:, bc:bc + 1], accum_out=sume)
                rec = small.tile([128, 1], F32, tag="rec")
                nc.vector.reciprocal(rec, sume)
                attn = small.tile([128, D], F32, tag="attn", bufs=1)
                nc.scalar.activation(attn, expt, AF.Copy, scale=rec)
                nc.sync.dma_start(out=attn_d[bc * 128:(bc + 1) * 128, :], in_=attn)
                outt = small.tile([128, D], F32, tag="outt", bufs=1)
                nc.vector.scalar_tensor_tensor(
                    out=outt, in0=attn, scalar=1.0, in1=ha_nat[:, bc, :],
                    op0=OP.add, op1=OP.mult)
                nc.sync.dma_start(out=out_d[bc * 128:(bc + 1) * 128, :], in_=outt)

        # direction l (h_a = l_hidden, h_b = m_hidden, G = G_l_m)
        phase1(gl_hi_d, gl_lo_d, haT_hi, haT_lo, hm_t, arg_l)
        phase2(arg_l, c_l, h1tl, 2, w2l_t, 0, ha_t, l_out_d, l_attn_d)
        # direction m
        phase1(gm_hi_d, gm_lo_d, hmT_hi, hmT_lo, ha_t, arg_m)
        phase2(arg_m, c_m, h1tm, 1, w2m_t, 1, hm_t, m_out_d, m_attn_d)

    nc.compile()
    return nc


_NC_CACHE = [None]


def get_program():
    if _NC_CACHE[0] is None:
        _NC_CACHE[0] = build_program()
    return _NC_CACHE[0]


def _split_bf16(x):
    hi = x.astype(ml_dtypes.bfloat16)
    lo = (x - hi.astype(np.float32)).astype(ml_dtypes.bfloat16)
    return hi, lo


def prepare_in_maps(l_hidden, m_hidden, G_l_m, G_m_l,
                    l_fc1_w, l_fc1_b, l_fc2_w, l_fc2_b,
                    m_fc1_w, m_fc1_b, m_fc2_w, m_fc2_b):
    ha = np.ascontiguousarray(np.asarray(l_hidden, np.float32).reshape(B, D))
    hm = np.ascontiguousarray(np.asarray(m_hidden, np.float32).reshape(B, D))
    haT = np.ascontiguousarray(ha.T)
    hmT = np.ascontiguousarray(hm.T)

    gl_hi, gl_lo = _split_bf16(np.asarray(G_l_m, np.float32).reshape(D, KD))
    gm_hi, gm_lo = _split_bf16(np.asarray(G_m_l, np.float32).reshape(D, KD))
    w1l_hi, w1l_lo = _split_bf16(np.ascontiguousarray(np.asarray(l_fc1_w, np.float32).T))
    w1m_hi, w1m_lo = _split_bf16(np.ascontiguousarray(np.asarray(m_fc1_w, np.float32).T))
    w2l = np.ascontiguousarray(np.asarray(l_fc2_w, np.float32).T)
    w2m = np.ascontiguousarray(np.asarray(m_fc2_w, np.float32).T)
    b1l = np.ascontiguousarray(np.asarray(l_fc1_b, np.float32).reshape(2, 128).T)
    b1m = np.ascontiguousarray(np.asarray(m_fc1_b, np.float32).reshape(1, 128).T)
    b2 = np.stack([np.asarray(l_fc2_b, np.float32),
                   np.asarray(m_fc2_b, np.float32)])

    shared = dict(gl_hi=gl_hi, gl_lo=gl_lo, gm_hi=gm_hi, gm_lo=gm_lo,
                  w1l_hi=w1l_hi, w1l_lo=w1l_lo, w1m_hi=w1m_hi, w1m_lo=w1m_lo,
                  w2l=w2l, w2m=w2m, b1l=b1l, b1m=b1m, b2=b2)

    in_maps = []
    for c in range(NC_CORES):
        s = slice(c * BC, (c + 1) * BC)
        hi_a, lo_a = _split_bf16(haT[:, s])
        hi_m, lo_m = _split_bf16(hmT[:, s])
        in_maps.append(dict(
            haT_hi=np.ascontiguousarray(hi_a), haT_lo=np.ascontiguousarray(lo_a),
            hmT_hi=np.ascontiguousarray(hi_m), hmT_lo=np.ascontiguousarray(lo_m),
            ha=ha[s], hm=hm[s], **shared))
    return in_maps


def kernel(**inputs):
    nc = get_program()
    in_maps = prepare_in_maps(**inputs)
    res = bass_utils.run_bass_kernel_spmd(
        nc, in_maps, core_ids=list(range(NC_CORES)))
    outs = {}
    for name in ("l_out", "m_out", "l_attn", "m_attn"):
        full = np.concatenate([res.results[c][name] for c in range(NC_CORES)], axis=0)
        outs[name] = full.reshape(B, 1, D).astype(np.float32)
    return outs["l_out"], outs["m_out"], outs["l_attn"], outs["m_attn"]


# revision 15
# speedup vs baseline: 1.0279x; 1.0279x over previous
"""CrossShareUnit TRN2 kernel — 8-core data-parallel Bass/Tile implementation.

Math (B=8192, L=1, D=1024, K=8):
    G_vec_a = (h_a @ W1.T + b1) @ W2.T + b2                     # [B, D]
    arg[b,k] = sum_{d,e} h_a[b,d] * G[d,k,e] * h_b[b,e]
    c[b]     = sum_k tanh(arg[b,k])
    attn     = softmax(c[b] * G_vec_a[b,:], axis=-1)
    out      = h_a + attn * h_a
for both directions (l->m with G_l_m, m->l with G_m_l).

Per core (batch shard of 1024):
  - s = h_aT.T @ G2 (G2 = reshape(G, [1024, 8192])) on TensorE as a 3-pass
    bf16 hi/lo split (hi*hi + hi*lo + lo*hi), exact to ~2^-17 relative.
  - arg via fused DVE tensor_tensor_reduce of the PSUM s-tile against the
    fp32 h_b rows; tanh + sum_k in one ScalarE activation with accum_out.
  - MLP: fc1 as 3-pass bf16 (reusing the h_aT splits), fc2 in plain fp32.
  - softmax fused on the fc2 PSUM tile (no max-subtract needed: |logits|<=10),
    residual via one scalar_tensor_tensor.
"""
import os
import sys

for _p in ("/opt/trn_rl_repo", "/root/.axon_site/_ro/trn_rl_repo"):
    if os.path.isdir(_p) and _p not in sys.path:
        sys.path.append(_p)

from contextlib import ExitStack

import numpy as np
import ml_dtypes

import concourse.bass as bass
import concourse.tile as tile
from concourse import bacc, mybir, bass_utils

BF16 = mybir.dt.bfloat16
F32 = mybir.dt.float32
AF = mybir.ActivationFunctionType
OP = mybir.AluOpType

B, D, K = 8192, 1024, 8
NC_CORES = 8
BC = B // NC_CORES          # 1024 batch rows per core
NB = BC // 128              # 8 batch chunks of 128
DC = D // 128               # 8 contraction chunks
KD = K * D                  # 8192
NN = KD // 512              # 16 G column blocks of 512
H_L, H_M = 256, 128         # fc1 hidden sizes


def _emit_fc1(nc, fc1ps, w1_hi, w1_lo, aT_hi, aT_lo, h1t, b1_t, n_hc):
    """h1T[h, b] = W1T.T @ h_aT + b1, 3-pass bf16. h1t: [128, n_hc, BC]."""
    for hc in range(n_hc):
        for bh in range(BC // 512):
            p = fc1ps.tile([128, 512], F32)
            first = True
            for c in range(DC):
                w_hi = w1_hi[:, c, hc * 128:(hc + 1) * 128]
                w_lo = w1_lo[:, c, hc * 128:(hc + 1) * 128]
                a_hi = aT_hi[:, c, bh * 512:(bh + 1) * 512]
                a_lo = aT_lo[:, c, bh * 512:(bh + 1) * 512]
                last = c == DC - 1
                nc.tensor.matmul(p, w_hi, a_hi, start=first, stop=False)
                nc.tensor.matmul(p, w_hi, a_lo, start=False, stop=False)
                nc.tensor.matmul(p, w_lo, a_hi, start=False, stop=last)
                first = False
            # psum + per-partition bias -> SBUF
            nc.scalar.activation(
                h1t[:, hc, bh * 512:(bh + 1) * 512], p, AF.Identity,
                bias=b1_t[:, hc:hc + 1], scale=1.0)


def build_program():
    nc = bacc.Bacc("TRN2", target_bir_lowering=False, debug=False,
                   num_devices=NC_CORES)

    def din(name, shape, dt):
        return nc.dram_tensor(name, shape, dt, kind="ExternalInput").ap()

    def dout(name, shape):
        return nc.dram_tensor(name, shape, F32, kind="ExternalOutput").ap()

    # per-core sharded inputs
    haT_hi_d = din("haT_hi", [D, BC], BF16)
    haT_lo_d = din("haT_lo", [D, BC], BF16)
    hmT_hi_d = din("hmT_hi", [D, BC], BF16)
    hmT_lo_d = din("hmT_lo", [D, BC], BF16)
    ha_d = din("ha", [BC, D], F32)
    hm_d = din("hm", [BC, D], F32)
    # replicated
    gl_hi_d = din("gl_hi", [D, KD], BF16)
    gl_lo_d = din("gl_lo", [D, KD], BF16)
    gm_hi_d = din("gm_hi", [D, KD], BF16)
    gm_lo_d = din("gm_lo", [D, KD], BF16)
    w1l_hi_d = din("w1l_hi", [D, H_L], BF16)
    w1l_lo_d = din("w1l_lo", [D, H_L], BF16)
    w1m_hi_d = din("w1m_hi", [D, H_M], BF16)
    w1m_lo_d = din("w1m_lo", [D, H_M], BF16)
    w2l_d = din("w2l", [H_L, D], mybir.dt.float32r)
    w2m_d = din("w2m", [H_M, D], mybir.dt.float32r)
    b1l_d = din("b1l", [128, 2], F32)
    b1m_d = din("b1m", [128, 1], F32)
    b2_d = din("b2", [2, D], F32)

    l_out_d = dout("l_out", [BC, D])
    m_out_d = dout("m_out", [BC, D])
    l_attn_d = dout("l_attn", [BC, D])
    m_attn_d = dout("m_attn", [BC, D])

    with tile.TileContext(nc) as tc, ExitStack() as ctx:
        resid = ctx.enter_context(tc.tile_pool(name="resid", bufs=1))
        small = ctx.enter_context(tc.tile_pool(name="small", bufs=2))

        # ---- resident tensors ----
        # Emission order doubles as DMA-queue priority: fc1's dependencies
        # first (PE can start ~15us in), the 21MB bulk after phase-1-l is
        # queued.
        def load3(dst, src):
            nc.sync.dma_start(out=dst, in_=src.rearrange("(c p) n -> p c n", p=128))

        haT_hi = resid.tile([128, DC, BC], BF16, tag="haT_hi")
        haT_lo = resid.tile([128, DC, BC], BF16, tag="haT_lo")
        hmT_hi = resid.tile([128, DC, BC], BF16, tag="hmT_hi")
        hmT_lo = resid.tile([128, DC, BC], BF16, tag="hmT_lo")
        ha_t = resid.tile([128, NB, D], F32, tag="ha")
        hm_t = resid.tile([128, NB, D], F32, tag="hm")
        b1l_t = resid.tile([128, 2], F32, tag="b1l")
        b1m_t = resid.tile([128, 1], F32, tag="b1m")
        h1tl = resid.tile([128, H_L // 128, BC], mybir.dt.float32r, tag="h1tl")
        h1tm = resid.tile([128, H_M // 128, BC], mybir.dt.float32r, tag="h1tm")
        w2l_t = resid.tile([128, H_L // 128, D], mybir.dt.float32r, tag="w2l")
        w2m_t = resid.tile([128, H_M // 128, D], mybir.dt.float32r, tag="w2m")
        b2bc = resid.tile([128, 2, D], F32, tag="b2bc")
        arg_l = resid.tile([128, NB, K, 2], F32, tag="arg_l")
        arg_m = resid.tile([128, NB, K, 2], F32, tag="arg_m")
        c_l = resid.tile([128, NB], F32, tag="c_l")
        c_m = resid.tile([128, NB], F32, tag="c_m")

        # ---- fc1 for both directions (3-pass bf16) ----
        load3(haT_hi, haT_hi_d)
        load3(haT_lo, haT_lo_d)
        nc.sync.dma_start(out=b1l_t, in_=b1l_d)
        nc.sync.dma_start(out=b1m_t, in_=b1m_d)
        with tc.tile_pool(name="w1pool", bufs=1) as w1p, \
             tc.tile_pool(name="fc1ps", bufs=2, space="PSUM") as fc1ps:
            w1l_hi = w1p.tile([128, DC, H_L], BF16, tag="w1l_hi")
            w1l_lo = w1p.tile([128, DC, H_L], BF16, tag="w1l_lo")
            w1m_hi = w1p.tile([128, DC, H_M], BF16, tag="w1m_hi")
            w1m_lo = w1p.tile([128, DC, H_M], BF16, tag="w1m_lo")
            load3(w1l_hi, w1l_hi_d)
            load3(w1l_lo, w1l_lo_d)
            _emit_fc1(nc, fc1ps, w1l_hi, w1l_lo, haT_hi, haT_lo, h1tl, b1l_t, 2)
            load3(hmT_hi, hmT_hi_d)
            load3(hmT_lo, hmT_lo_d)
            load3(w1m_hi, w1m_hi_d)
            load3(w1m_lo, w1m_lo_d)
            _emit_fc1(nc, fc1ps, w1m_hi, w1m_lo, hmT_hi, hmT_lo, h1tm, b1m_t, 1)

        # needed by phase-1-l's reductions ~45us in
        load3(hm_t, hm_d)

        gpool = ctx.enter_context(tc.tile_pool(name="gpool", bufs=2))
        spsum = ctx.enter_context(tc.tile_pool(name="spsum", bufs=3, space="PSUM"))
        gvps = ctx.enter_context(tc.tile_pool(name="gvps", bufs=2, space="PSUM"))

        def load_bulk():
            load3(ha_t, ha_d)
            load3(w2l_t, w2l_d)
            load3(w2m_t, w2m_d)
            # b2 rows broadcast across partitions (DVE rejects stride-0 APs)
            for r in range(2):
                row = b2_d[r:r + 1, :]
                src = bass.AP(tensor=row.tensor, offset=row.offset,
                              ap=[[0, 128]] + [list(a) for a in row.ap[1:]])
                nc.sync.dma_start(out=b2bc[:, r, :], in_=src)

        def phase1(g_hi_d, g_lo_d, aT_hi, aT_lo, hb_t, arg_t, after_n=None):
            """s = aT.T @ G2 (3-pass);  arg[b, k, j] = <s_tile, hb_slice>."""
            for n in range(NN):
                if after_n is not None and n == 3:
                    after_n()
                k, j = n // 2, n % 2
                ghi = gpool.tile([128, DC, 512], BF16, tag="ghi")
                glo = gpool.tile([128, DC, 512], BF16, tag="glo")
                src_hi = g_hi_d[:, n * 512:(n + 1) * 512]
                src_lo = g_lo_d[:, n * 512:(n + 1) * 512]
                nc.sync.dma_start(out=ghi, in_=src_hi.rearrange("(c p) n -> p c n", p=128))
                nc.sync.dma_start(out=glo, in_=src_lo.rearrange("(c p) n -> p c n", p=128))
                for bc in range(NB):
                    p = spsum.tile([128, 512], F32)
                    first = True
                    for c in range(DC):
                        a_hi = aT_hi[:, c, bc * 128:(bc + 1) * 128]
                        a_lo = aT_lo[:, c, bc * 128:(bc + 1) * 128]
                        last = c == DC - 1
                        nc.tensor.matmul(p, a_hi, ghi[:, c, :], start=first, stop=False)
                        nc.tensor.matmul(p, a_hi, glo[:, c, :], start=False, stop=False)
                        nc.tensor.matmul(p, a_lo, ghi[:, c, :], start=False, stop=last)
                        first = False
                    # arg[b, k, j] = sum_e s[b, e] * h_b[b, e]; product scratch
                    # written back over the psum tile (no SBUF scratch needed).
                    # scalar_tensor_tensor, NOT tensor_tensor_reduce: the TTR
                    # opcode hard-crashes the exec unit on this hw/ucode.
                    nc.vector.scalar_tensor_tensor(
                        out=p, in0=p, scalar=1.0,
                        in1=hb_t[:, bc, j * 512:(j + 1) * 512],
                        op0=OP.mult, op1=OP.mult,
                        accum_out=arg_t[:, bc, k, j:j + 1])

        def phase2(arg_t, c_t, h1t, n_hc, w2_t, b2_row, ha_nat, out_d, attn_d):
            for bc in range(NB):
                targ = small.tile([128, K], F32, tag="targ")
                nc.vector.tensor_reduce(
                    out=targ, in_=arg_t[:, bc], axis=mybir.AxisListType.X,
                    op=OP.add)
                tscr = small.tile([128, K], F32, tag="tscr")
                nc.scalar.activation(tscr, targ, AF.Tanh,
                                     accum_out=c_t[:, bc:bc + 1])
                # fc2: Gv[b, d] = h1T.T @ W2T  (fp32), one 2-bank psum row
                gv = gvps.tile([128, D], F32)
                for dh in range(2):
                    for hc in range(n_hc):
                        nc.tensor.matmul(
                            gv[:, dh * 512:(dh + 1) * 512],
                            h1t[:, hc, bc * 128:(bc + 1) * 128],
                            w2_t[:, hc, dh * 512:(dh + 1) * 512],
                            start=(hc == 0), stop=(hc == n_hc - 1))
                nc.vector.tensor_tensor(out=gv, in0=gv, in1=b2bc[:, b2_row, :],
                                        op=OP.add)
                expt = small.tile([128, D], F32, tag="expt", bufs=1)
                sume = small.tile([128, 1], F32, tag="sume")
                nc.scalar.activation(expt, gv, AF.Exp,
                                     scale=c_t[:, bc:bc + 1], accum_out=sume)
                rec = small.tile([128, 1], F32, tag="rec")
                nc.vector.reciprocal(rec, sume)
                attn = small.tile([128, D], F32, tag="attn", bufs=1)
                nc.scalar.activation(attn, expt, AF.Copy, scale=rec)
                nc.sync.dma_start(out=attn_d[bc * 128:(bc + 1) * 128, :], in_=attn)
                outt = small.tile([128, D], F32, tag="outt", bufs=1)
                nc.vector.scalar_tensor_tensor(
                    out=outt, in0=attn, scalar=1.0, in1=ha_nat[:, bc, :],
                    op0=OP.add, op1=OP.mult)
                nc.sync.dma_start(out=out_d[bc * 128:(bc + 1) * 128, :], in_=outt)

        # direction l (h_a = l_hidden, h_b = m_hidden, G = G_l_m)
        phase1(gl_hi_d, gl_lo_d, haT_hi, haT_lo, hm_t, arg_l, after_n=load_bulk)
        phase2(arg_l, c_l, h1tl, 2, w2l_t, 0, ha_t, l_out_d, l_attn_d)
        # direction m
        phase1(gm_hi_d, gm_lo_d, hmT_hi, hmT_lo, ha_t, arg_m)
        phase2(arg_m, c_m, h1tm, 1, w2m_t, 1, hm_t, m_out_d, m_attn_d)

    nc.compile()
    return nc


_NC_CACHE = [None]


def get_program():
    if _NC_CACHE[0] is None:
        _NC_CACHE[0] = build_program()
    return _NC_CACHE[0]


def _split_bf16(x):
    hi = x.astype(ml_dtypes.bfloat16)
    lo = (x - hi.astype(np.float32)).astype(ml_dtypes.bfloat16)
    return hi, lo


def _round_f32r(x):
    """Round fp32 to E8M11 (the fp32r matmul operand format)."""
    u = np.ascontiguousarray(x.astype(np.float32)).view(np.uint32)
    u = (u.astype(np.uint64) + 0x800) & 0xFFFFF000
    return u.astype(np.uint32).view(np.float32)


def prepare_in_maps(l_hidden, m_hidden, G_l_m, G_m_l,
                    l_fc1_w, l_fc1_b, l_fc2_w, l_fc2_b,
                    m_fc1_w, m_fc1_b, m_fc2_w, m_fc2_b):
    ha = np.ascontiguousarray(np.asarray(l_hidden, np.float32).reshape(B, D))
    hm = np.ascontiguousarray(np.asarray(m_hidden, np.float32).reshape(B, D))
    haT = np.ascontiguousarray(ha.T)
    hmT = np.ascontiguousarray(hm.T)

    gl_hi, gl_lo = _split_bf16(np.asarray(G_l_m, np.float32).reshape(D, KD))
    gm_hi, gm_lo = _split_bf16(np.asarray(G_m_l, np.float32).reshape(D, KD))
    w1l_hi, w1l_lo = _split_bf16(np.ascontiguousarray(np.asarray(l_fc1_w, np.float32).T))
    w1m_hi, w1m_lo = _split_bf16(np.ascontiguousarray(np.asarray(m_fc1_w, np.float32).T))
    w2l = _round_f32r(np.ascontiguousarray(np.asarray(l_fc2_w, np.float32).T))
    w2m = _round_f32r(np.ascontiguousarray(np.asarray(m_fc2_w, np.float32).T))
    b1l = np.ascontiguousarray(np.asarray(l_fc1_b, np.float32).reshape(2, 128).T)
    b1m = np.ascontiguousarray(np.asarray(m_fc1_b, np.float32).reshape(1, 128).T)
    b2 = np.stack([np.asarray(l_fc2_b, np.float32),
                   np.asarray(m_fc2_b, np.float32)])

    shared = dict(gl_hi=gl_hi, gl_lo=gl_lo, gm_hi=gm_hi, gm_lo=gm_lo,
                  w1l_hi=w1l_hi, w1l_lo=w1l_lo, w1m_hi=w1m_hi, w1m_lo=w1m_lo,
                  w2l=w2l, w2m=w2m, b1l=b1l, b1m=b1m, b2=b2)

    in_maps = []
    for c in range(NC_CORES):
        s = slice(c * BC, (c + 1) * BC)
        hi_a, lo_a = _split_bf16(haT[:, s])
        hi_m, lo_m = _split_bf16(hmT[:, s])
        in_maps.append(dict(
            haT_hi=np.ascontiguousarray(hi_a), haT_lo=np.ascontiguousarray(lo_a),
            hmT_hi=np.ascontiguousarray(hi_m), hmT_lo=np.ascontiguousarray(lo_m),
            ha=ha[s], hm=hm[s], **shared))
    return in_maps


def kernel(**inputs):
    nc = get_program()
    in_maps = prepare_in_maps(**inputs)
    res = bass_utils.run_bass_kernel_spmd(
        nc, in_maps, core_ids=list(range(NC_CORES)))
    outs = {}
    for name in ("l_out", "m_out", "l_attn", "m_attn"):
        full = np.concatenate([res.results[c][name] for c in range(NC_CORES)], axis=0)
        outs[name] = full.reshape(B, 1, D).astype(np.float32)
    return outs["l_out"], outs["m_out"], outs["l_attn"], outs["m_attn"]
